# revision 4
# baseline (speedup 1.0000x reference)
"""GAT layer kernel for Trainium2, 8 NeuronCores, data-parallel (v2).

Problem: nn_GATLayer (B=4, N=2048, F_IN=64, F_OUT=64, H=4).

Sharding: core c handles batch b = c//2 and destination-node rows
[ (c%2)*1024, (c%2)*1024+1024 ) of that batch.  Inputs are rolled per
core so the core's destination rows sit at x[0:NI] (adj columns rolled
to match); the j-permutation is softmax-invariant.

v2 reformulation — no per-element exp, no custom DVE op:
  e_ji = exp(lrelu(u_i + v_j)) = E_u[i] * E_v[j] * max(1, R_u[i]*R_v[j])
  with E = exp(.), R = exp(-0.8 .).  E_u cancels in softmax (num/den);
  E_v folds into the matmul stationary (haug = [h*E_v | E_v]).  The
  per-element work is only
     t = max(R_u_bcast * R_v, 1)        (tensor_scalar, 4x DVE mode)
     M = t * adjT                       (tensor_tensor, 2x DVE mode)
  num/den = haug^T @ M accumulated over j-chunks in PSUM.

Mask path: adjacency int32 rows are loaded once, bitcast to float16
(low halves hold 0x0000/0x0001 = 0 / subnormal 5.96e-8; the uniform
scale cancels in the softmax ratio), PE-transposed per
128x128 tile straight from the strided u16 view into u16 PSUM, and
copied (with convert) to a bf16 adjT in SBUF.  No DRAM round trip.

The main loop runs in i-halves: the left half of the i range only needs
adjacency row-chunks 0..3, so compute starts when half the adjacency
has arrived.
"""

import sys

sys.path.insert(0, "/opt/trn_rl_repo")

from contextlib import ExitStack

import numpy as np

import concourse.bass as bass
import concourse.mybir as mybir
import concourse.tile as tile
from concourse import bacc
from concourse.bass_utils import run_bass_kernel_spmd
from concourse.masks import make_identity

F32 = mybir.dt.float32
BF16 = mybir.dt.bfloat16
I32 = mybir.dt.int32
U16 = mybir.dt.uint16
F16 = mybir.dt.float16
ALU = mybir.AluOpType
ACTF = mybir.ActivationFunctionType

B, N, F_IN, F_OUT, H = 4, 2048, 64, 64, 4
NI = N // 2            # destination rows per core
P = 128                # partitions
NJC = N // P           # 16 j-chunks
NIT = NI // P          # 8 i-tiles (per-core rows / 128)
HO = H * F_OUT         # 256
NH = 512               # i-half width in columns

POOL_EVERY = 4         # every POOL_EVERY-th mask-multiply runs on Pool


def gat_core_program(tc, outs, ins):
    nc = tc.nc
    ctx = ExitStack()
    x_d, adj_d, w_d, attn_d = ins["x"], ins["adj"], ins["w"], ins["attn"]
    out_d = outs["out"]

    const = ctx.enter_context(tc.tile_pool(name="const", bufs=1))

    # ---------------- persistent tensors ----------------
    ident = const.tile([P, P], F32)
    make_identity(nc, ident[:])
    identh = const.tile([P, P], F16)
    nc.vector.tensor_copy(identh[:], ident[:])

    adjT = const.tile([P, NJC, NI], BF16)         # transposed 0/1 mask
    rub = const.tile([P, H, NI], BF16)            # R_u broadcast
    haug = const.tile([P, NJC, H, F_OUT + 1], BF16)  # [h | 1]
    rvev = const.tile([P, NJC, H], F32)           # exp(0.2 v) (j on parts)
    evsc = const.tile([P, NJC, H], F32)           # exp(v)     (j on parts)
    outf = const.tile([P, NIT, HO], F32)          # final output staging

    mctx = ExitStack()
    adj_pool = mctx.enter_context(tc.tile_pool(name="adjld", bufs=8))
    mask_ps = mctx.enter_context(tc.tile_pool(name="mask_ps", bufs=2,
                                              space="PSUM"))
    sctx = ExitStack()
    setup_sb = sctx.enter_context(tc.tile_pool(name="setup_sb", bufs=1))
    setup_ps = sctx.enter_context(tc.tile_pool(name="setup_ps", bufs=2, space="PSUM"))

    # ---------------- DMA issues ----------------
    w_sb = const.tile([F_IN, HO], F32)
    nc.sync.dma_start(w_sb[:], w_d[:])
    x_sb = setup_sb.tile([P, NJC, F_IN], F32, tag="xload")
    nc.sync.dma_start(x_sb[:], x_d.rearrange("(s p) c -> p s c", p=P))

    aa = setup_sb.tile([P, 2, 2 * H], F32)
    nc.gpsimd.memset(aa[:], 0.0)
    for h in range(H):
        half, poff = divmod(h * F_OUT, P)
        nc.sync.dma_start(
            aa[poff:poff + F_OUT, half, h],
            attn_d[h, 0:F_OUT].rearrange("(a f) -> f a", a=1))
        nc.sync.dma_start(
            aa[poff:poff + F_OUT, half, H + h],
            attn_d[h, F_OUT:2 * F_OUT].rearrange("(a f) -> f a", a=1))

    # adjacency row-chunk loads on the Pool queue (cheap issue, big data)
    adj_blk = adj_d.rearrange("(s p) c -> p s c", p=P)
    a_tiles = []
    for it in range(NIT):
        a_i32 = adj_pool.tile([P, 1, N], I32, name=f"a{it}", tag="a_i32")
        nc.sync.dma_start(a_i32[:], adj_blk[:, it:it + 1, :])
        a_tiles.append(a_i32)

    # ---------------- setup: xT, W^T, attention products ----------------
    xT = const.tile([F_IN, N], F32)
    for s in range(NJC):
        pt = setup_ps.tile([F_IN, P], F32, tag="sps")
        nc.tensor.transpose(pt[:], x_sb[:, s, :], ident[:])
        nc.vector.tensor_copy(xT[:, s * P:(s + 1) * P], pt[:])

    wT = setup_sb.tile([P, 2, F_IN], F32)
    for half in range(2):
        pt = setup_ps.tile([P, F_IN], F32, tag="sps")
        nc.tensor.transpose(pt[:], w_sb[:, half * P:(half + 1) * P],
                            ident[:F_IN, :F_IN])
        nc.scalar.copy(wT[:, half, :], pt[:])

    wa = const.tile([F_IN, 2 * H], F32)
    pwa = setup_ps.tile([F_IN, 2 * H], F32, tag="sps")
    for half in range(2):
        nc.tensor.matmul(pwa[:], wT[:, half, :], aa[:, half, :],
                         start=(half == 0), stop=(half == 1))
    nc.vector.tensor_copy(wa[:], pwa[:])

    # uvT over full x: rows 0:4 -> u-heads, 4:8 -> v-heads.  [8, N]
    uvT = setup_sb.tile([2 * H, N], F32)
    for ch in range(N // 512):
        pv = setup_ps.tile([2 * H, 512], F32, tag="sps")
        nc.tensor.matmul(pv[:], wa[:], xT[:, ch * 512:(ch + 1) * 512],
                         start=True, stop=True)
        nc.vector.tensor_copy(uvT[:, ch * 512:(ch + 1) * 512], pv[:])

    # R_u rows for this core's i-range (rows 0..NI after the roll);
    # DMA-reshaped onto partition 0 so they can be a matmul moving operand
    ru_rows = setup_sb.tile([H, NI], F32)
    nc.scalar.activation(ru_rows[:], uvT[0:H, 0:NI], ACTF.Exp, scale=-0.8)
    ru0 = setup_sb.tile([1, H, NI], F32)
    nc.sync.dma_start(ru0[:], ru_rows[:])

    # rvev = exp(0.2 v), evsc = exp(v), j on partitions (batched exps)
    uvtt = setup_ps.tile([P, NJC, 2 * H], F32, tag="uvtt")
    for jc in range(NJC):
        nc.tensor.transpose(uvtt[:, jc, :], uvT[:, jc * P:(jc + 1) * P],
                            ident[:2 * H, :2 * H])
    nc.scalar.activation(rvev[:], uvtt[:, :, H:2 * H], ACTF.Exp, scale=0.2)
    nc.scalar.activation(evsc[:], uvtt[:, :, H:2 * H], ACTF.Exp)

    # R_u broadcast across partitions (512-wide psum chunks)
    ones_row = setup_sb.tile([1, P], F32)
    nc.gpsimd.memset(ones_row[:], 1.0)
    for h in range(H):
        for ch in range(NI // 512):
            pb = setup_ps.tile([P, 512], F32, tag="spsb")
            nc.tensor.matmul(pb[:], ones_row[:],
                             ru0[:, h, ch * 512:(ch + 1) * 512],
                             start=True, stop=True)
            if (h + ch) % 2 == 0:
                nc.vector.tensor_copy(rub[:, h, ch * 512:(ch + 1) * 512],
                                      pb[:])
            else:
                nc.scalar.copy(rub[:, h, ch * 512:(ch + 1) * 512], pb[:])

    # h = x @ W; haug[:, s, h, 0:64] = h_s;  haug[:, s, h, 64] = 1
    haug3 = haug[:].rearrange("p s h f -> p (s h) f")
    nc.gpsimd.memset(haug3[:, :, F_OUT:F_OUT + 1], 1.0)

    def build_haug():
        for s in range(NJC):
            ph = setup_ps.tile([P, HO], F32, tag="sps")
            nc.tensor.matmul(ph[:], xT[:, s * P:(s + 1) * P], w_sb[:],
                             start=True, stop=True)
            nc.scalar.copy(
                haug[:, s, :, 0:F_OUT],
                ph[:].rearrange("p (h f) -> p h f", h=H))

    # ---------------- mask transposes (PE, u16 view) ----------------

    def mask_chunk(it):
        a_f16 = a_tiles[it][:].bitcast(F16)  # [P, 1, 2N]
        for half in range(2):
            pm = mask_ps.tile([P, 8, P], F16, tag="pm")
            for j8 in range(8):
                jc = half * 8 + j8
                src = a_f16[:, 0, jc * 2 * P:(jc + 1) * 2 * P:2]
                nc.tensor.transpose(pm[:, j8, :], src, identh[:])
            dst = adjT[:, half * 8:(half + 1) * 8, it * P:(it + 1) * P]
            if (2 * it + half) % 2 == 0:
                nc.scalar.copy(dst, pm[:])
            else:
                nc.vector.tensor_copy(dst, pm[:])

    # ---------------- main loop ----------------

    unit = [0]

    def sweep(hp, mh, pos):
        csl = slice(mh * NH, (mh + 1) * NH)
        for jc in range(NJC):
            t2 = tpool.tile([P, 2, NH], BF16, tag="t")
            for i in range(2):
                h = hp * 2 + i
                nc.vector.tensor_scalar(
                    t2[:, i, :], rub[:, h, csl], rvev[:, jc, h:h + 1],
                    evsc[:, jc, h:h + 1], op0=ALU.mult, op1=ALU.max)
            m2 = mpool.tile([P, 2, NH], BF16, tag="m")
            adj_b = adjT[:, jc, csl].unsqueeze(1).broadcast_to([P, 2, NH])
            eng = (nc.gpsimd if unit[0] % POOL_EVERY == POOL_EVERY - 1
                   else nc.vector)
            eng.tensor_tensor(m2[:], t2[:], adj_b, op=ALU.mult)
            unit[0] += 1
            for i in range(2):
                h = hp * 2 + i
                nc.tensor.matmul(
                    pos[i][:, csl], haug[:, jc, h, :], m2[:, i, :],
                    start=(jc == 0), stop=(jc == NJC - 1))

    def epilogue(hp, pos):
        for i in range(2):
            h = hp * 2 + i
            ot = ot_pool.tile([F_OUT + 1, NI], F32)
            nc.scalar.copy(ot[:], pos[i][:])
            for it in range(NIT):
                ptp = pt_pool.tile([P, F_OUT + 1], F32)
                nc.tensor.transpose(ptp[:], ot[:, it * P:(it + 1) * P],
                                    ident[:F_OUT + 1, :F_OUT + 1])
                rec = rec_pool.tile([P, 1], F32)
                nc.vector.reciprocal(rec[:], ptp[:, F_OUT:F_OUT + 1])
                nc.scalar.activation(
                    outf[:, it, h * F_OUT:(h + 1) * F_OUT],
                    ptp[:, 0:F_OUT], ACTF.Copy, scale=rec[:])

    mask_chunk(0)
    mask_chunk(1)
    build_haug()
    sctx.close()

    ctx2 = ExitStack()
    tpool = ctx2.enter_context(tc.tile_pool(name="twork", bufs=3))
    mpool = ctx2.enter_context(tc.tile_pool(name="mwork", bufs=3))
    po_pool = ctx2.enter_context(tc.tile_pool(name="po", bufs=1, space="PSUM"))
    pt_pool = ctx2.enter_context(tc.tile_pool(name="ptrans", bufs=2, space="PSUM"))
    ot_pool = ctx2.enter_context(tc.tile_pool(name="otsb", bufs=2))
    rec_pool = ctx2.enter_context(tc.tile_pool(name="rec", bufs=2))
    pos0 = [po_pool.tile([F_OUT + 1, NI], F32, name=f"po0_{i}",
                         tag=f"po{i}") for i in range(2)]
    mask_chunk(2)
    mask_chunk(3)
    sweep(0, 0, pos0)
    for it in range(NIT // 2, NIT):
        mask_chunk(it)
    sweep(0, 1, pos0)
    epilogue(0, pos0)

    pos1 = [po_pool.tile([F_OUT + 1, NI], F32, name=f"po1_{i}",
                         tag=f"po{i}") for i in range(2)]
    sweep(1, 0, pos1)
    sweep(1, 1, pos1)
    epilogue(1, pos1)

    for it in range(NIT):
        nc.sync.dma_start(
            out_d.rearrange("(s p) c -> p s c", p=P)[:, it, :],
            outf[:, it, :])
    ctx2.close()
    mctx.close()
    ctx.close()


N_CORES = 8
_CACHE = {}


def _build(repeats=1):
    key = ("nc", repeats)
    if key not in _CACHE:
        nc = bacc.Bacc("TRN2", target_bir_lowering=False, debug=False,
                       num_devices=N_CORES)
        ins = {
            "x": nc.dram_tensor("x", [N, F_IN], F32, kind="ExternalInput").ap(),
            "adj": nc.dram_tensor("adj", [NI, N], I32, kind="ExternalInput").ap(),
            "w": nc.dram_tensor("w", [F_IN, HO], F32,
                                kind="ExternalInput").ap(),
            "attn": nc.dram_tensor("attn", [H, 2 * F_OUT], F32,
                                   kind="ExternalInput").ap(),
        }
        outs = {"out": nc.dram_tensor("out", [NI, HO], F32,
                                      kind="ExternalOutput").ap()}
        with tile.TileContext(nc) as tc:
            for _ in range(repeats):
                gat_core_program(tc, outs, ins)
        nc.compile()
        _CACHE[key] = nc
    return _CACHE[key]


def make_in_maps(node_features, adj_matrix, W, attention):
    node_features = np.ascontiguousarray(node_features, dtype=np.float32)
    adj_matrix = np.ascontiguousarray(adj_matrix, dtype=np.int32)
    W = np.ascontiguousarray(W, dtype=np.float32)
    attention = np.ascontiguousarray(attention, dtype=np.float32)
    in_maps = []
    for c in range(N_CORES):
        b, ih = divmod(c, 2)
        i0 = ih * NI
        # roll so this core's destination rows start at node 0; the same
        # j-permutation is applied to adjacency columns (softmax and the
        # weighted sum are invariant to a consistent j relabeling).
        xr = np.roll(node_features[b], -i0, axis=0)
        adjr = np.roll(adj_matrix[b, i0:i0 + NI], -i0, axis=1)
        in_maps.append({
            "x": np.ascontiguousarray(xr),
            "adj": np.ascontiguousarray(adjr),
            "w": W,
            "attn": attention,
        })
    return in_maps


def assemble(results):
    out = np.empty((B, N, HO), dtype=np.float32)
    for c in range(N_CORES):
        b, ih = divmod(c, 2)
        i0 = ih * NI
        out[b, i0:i0 + NI] = results[c]["out"]
    return out


def kernel(node_features, adj_matrix, W, attention):
    nc = _build()
    in_maps = make_in_maps(node_features, adj_matrix, W, attention)
    res = run_bass_kernel_spmd(nc, in_maps, core_ids=list(range(N_CORES)))
    return assemble(res.results)
